# revision 15
# baseline (speedup 1.0000x reference)
"""MoE (16 routed experts, top-2, + shared expert) on 8 Trainium2 cores.

Strategy (expert-parallel, host-side dispatch):
  - Host computes the gate (softmax + top-2) and gathers each expert's
    tokens; experts are permuted so each core owns one "big" and one
    "small" expert (slot capacities CA >= CB), minimizing pad columns.
  - The shared expert is split across core pairs: core c computes the
    hidden slice [q*HS/2, (q+1)*HS/2) (q = c%2) of the shared expert
    for the 512 tokens owned by its pair; the host adds the two
    halves. This halves the (otherwise 8x-replicated) shared-expert
    weight traffic per core.
  - Each core runs the same program: three gated-MLP blocks
    (expertA, expertB, shared-half) in a transposed layout
        zT = W2^T @ (u * silu(g)),  [u;g]^T = W1^T @ xT
    so no on-chip transposes are needed anywhere.
  - Routed expert weights travel as fp8e4 (e4m3) scaled by 128; the
    silu input is descaled on the ACT engine (activation scale=1/128)
    and the host divides the routed outputs by 128^2. The shared
    expert stays f16. End-to-end rel err ~1.2e-2 (gate is 2e-2).
  - Weights are packed flat per partition ([P, n]) so every DMA reads
    one contiguous run per partition; weight loads stream on the sync
    (SP) HWDGE FIFO in exactly the order compute consumes them, while
    x loads and z stores ride the scalar (ACT) FIFO.
"""

import sys

for _p in ("/opt/trn_rl_repo", "/root/.axon_site/_ro/trn_rl_repo"):
    if _p not in sys.path:
        sys.path.insert(0, _p)

import contextlib
import os

import numpy as np
import ml_dtypes

import concourse.bass as bass  # noqa: F401
import concourse.tile as tile
from concourse import bacc, mybir
from concourse.bass_utils import run_bass_kernel_spmd

try:  # tracing needs the axon NTFF hook; absent in some containers
    from antenv import axon_hooks as _axon_hooks  # noqa: F401
except ImportError:
    os.environ.setdefault("BASS_NEVER_TRACE", "1")

B, S, D = 2, 1024, 1024
H = 512           # routed expert hidden
HS = 1024         # shared expert hidden
HL = HS // 2      # shared hidden per core (pair-split)
E = 16
ROUTE_SCALE = 1.0
T = B * S
N_CORES = 8
TDP2 = 2 * (T // N_CORES)   # shared-expert tokens per core pair (512)
P = 128
KD = D // P                 # fc1 contraction chunks
MD = D // P                 # fc2 output chunks
NH = H // P                 # routed fc1 output chunks per half (4)
NHL = HL // P               # shared-half fc1 output chunks per half (4)
GH = 2                      # fc1 weight-chunk group (hc's per DMA)
GD = 4                      # fc2 weight-chunk group (dp's per DMA)

F32 = mybir.dt.float32
F16 = mybir.dt.float16
FP8 = mybir.dt.float8e4
NP16 = np.float16
NP8 = ml_dtypes.float8_e4m3fn
W8SCALE = 128.0             # fp8 weight pre-scale (per routed matmul)
ROUTED_FP8 = True
DR_FC1 = True               # DoubleRow (fp8 x + fp8 w1) on routed fc1
ACT = mybir.ActivationFunctionType
DROW = mybir.MatmulPerfMode.DoubleRow

LAST_RESULTS = None
_NC_CACHE = {}


def _build_nc(CA, CB, reps=1):
    """SPMD program: routed blocks (capacities CA, CB) + shared-half block."""
    nc = bacc.Bacc(None, target_bir_lowering=False)
    wdt_r = FP8 if ROUTED_FP8 else F16
    asc_r = (1.0 / W8SCALE) if ROUTED_FP8 else 1.0

    xdt_r = FP8 if (ROUTED_FP8 and DR_FC1) else F16
    dr_r = ROUTED_FP8 and DR_FC1
    rblocks = []
    for i, cap in enumerate((CA, CB)):
        rblocks.append((
            nc.declare_dram_parameter(f"w1e{i}", [P, NH * 2 * KD * P], wdt_r, isOutput=False),
            nc.declare_dram_parameter(f"w2e{i}", [P, MD * NH * P], wdt_r, isOutput=False),
            NH,
            nc.declare_dram_parameter(f"xg{i}", [P, KD * cap], xdt_r, isOutput=False),
            cap,
            nc.declare_dram_parameter(f"zg{i}", [P, MD * cap], F16, isOutput=True),
            wdt_r, asc_r, xdt_r, dr_r,
        ))
    sblock = (
        nc.declare_dram_parameter("ws1", [P, NHL * 2 * KD * P], F16, isOutput=False),
        nc.declare_dram_parameter("ws2", [P, MD * NHL * P], F16, isOutput=False),
        NHL,
        nc.declare_dram_parameter("xd", [P, KD * TDP2], F16, isOutput=False),
        TDP2,
        nc.declare_dram_parameter("zs", [P, MD * TDP2], F16, isOutput=True),
        F16, 1.0, F16, False,
    )
    # order: big expert first (small head), small expert last (small tail)
    blocks = [rblocks[0], sblock, rblocks[1]]

    with tile.TileContext(nc) as tc:
        with (
            tc.tile_pool(name="xpool", bufs=1) as xpool,
            tc.tile_pool(name="wpool", bufs=1) as wpool,
            tc.tile_pool(name="hpool", bufs=1) as hpool,
            tc.tile_pool(name="spool", bufs=2) as spool,
            tc.tile_pool(name="opool", bufs=1) as opool,
            tc.tile_pool(name="psu", bufs=3, space="PSUM") as psu,
            tc.tile_pool(name="psg", bufs=3, space="PSUM") as psg,
            tc.tile_pool(name="psz", bufs=2, space="PSUM") as psz,
        ):
            pools = (xpool, wpool, hpool, spool, opool, psu, psg, psz)
            # PE body > 256 instructions: hint the back-edge so the loop
            # branch I$-hits instead of stalling ~4us on an IRAM fetch
            loop_cm = (tc.For_i(0, reps, 1, hint_engines=(mybir.EngineType.PE,))
                       if reps > 1 else contextlib.nullcontext())
            with loop_cm:
                _emit_body(nc, blocks, pools)
    nc.finalize()
    return nc


def _emit_body(nc, blocks, pools):
    xpool, wpool, hpool, spool, opool, psu, psg, psz = pools
    last = len(blocks) - 1
    for bi, (w1, w2, NHb, xt, C, zt, wdt, ascale, xdt, dr) in enumerate(blocks):
        w1_a = w1.ap()
        w2_a = w2.ap()
        xt_a = xt.ap().rearrange("p (k c) -> p k c", k=KD)
        zt_a = zt.ap().rearrange("p (m c) -> p m c", m=MD)

        x_tile = xpool.tile([P, KD, C], xdt, tag=f"x{bi}")
        nc.scalar.dma_start(x_tile[:], xt_a)
        h_tile = hpool.tile([P, NHb, C], F16, tag=f"h{bi}")

        # fc1 weight chunks, in consumption order on the sync FIFO.
        # Block 0 uses single-hc chunks so the PE starts sooner.
        gh = 1 if bi == 0 else GH
        csz1 = gh * 2 * KD * P
        w1ts = []
        for cc in range(NHb // gh):
            if dr:
                w1t = wpool.tile([P, gh, 2, KD // 2, 2, P], wdt, tag=f"w1_{bi}_{cc}")
                w1t_src = w1_a[:, cc * csz1:(cc + 1) * csz1].rearrange(
                    "p (h s j t f) -> p h s j t f", h=gh, s=2, j=KD // 2, t=2)
            else:
                w1t = wpool.tile([P, gh, 2, KD, P], wdt, tag=f"w1_{bi}_{cc}")
                w1t_src = w1_a[:, cc * csz1:(cc + 1) * csz1].rearrange(
                    "p (h s k f) -> p h s k f", h=gh, s=2, k=KD)
            nc.sync.dma_start(w1t[:], w1t_src)
            w1ts.append(w1t)

        for hc in range(NHb):
            w1t = w1ts[hc // gh]
            g = hc % gh
            for c0 in range(0, C, 512):
                c1 = min(C, c0 + 512)
                ps_u = psu.tile([P, c1 - c0], F32, tag="psu")
                ps_g = psg.tile([P, c1 - c0], F32, tag="psg")
                if dr:
                    for j in range(KD // 2):
                        nc.tensor.matmul(ps_u[:], w1t[:, g, 0, j],
                                         x_tile[:, 2 * j:2 * j + 2, c0:c1],
                                         start=(j == 0), stop=(j == KD // 2 - 1),
                                         perf_mode=DROW)
                    for j in range(KD // 2):
                        nc.tensor.matmul(ps_g[:], w1t[:, g, 1, j],
                                         x_tile[:, 2 * j:2 * j + 2, c0:c1],
                                         start=(j == 0), stop=(j == KD // 2 - 1),
                                         perf_mode=DROW)
                else:
                    for k in range(KD):
                        nc.tensor.matmul(ps_u[:], w1t[:, g, 0, k], x_tile[:, k, c0:c1],
                                         start=(k == 0), stop=(k == KD - 1))
                    for k in range(KD):
                        nc.tensor.matmul(ps_g[:], w1t[:, g, 1, k], x_tile[:, k, c0:c1],
                                         start=(k == 0), stop=(k == KD - 1))
                sil = spool.tile([P, c1 - c0], F32, tag="sil")
                if ascale != 1.0:
                    nc.scalar.activation(sil[:], ps_g[:], ACT.Silu, scale=ascale)
                else:
                    nc.scalar.activation(sil[:], ps_g[:], ACT.Silu)
                nc.vector.tensor_mul(h_tile[:, hc, c0:c1], ps_u[:], sil[:])

        o_tile = opool.tile([P, MD, C], F16, tag=f"o{bi}")
        csz2 = GD * NHb * P
        for cc in range(MD // GD):
            w2t = wpool.tile([P, GD, NHb, P], wdt, tag=f"w2_{bi}_{cc}")
            # fc2 weights ride the scalar FIFO: spreads load bandwidth
            # across both HWDGE rings and keeps w1 chunks back-to-back
            nc.scalar.dma_start(
                w2t[:],
                w2_a[:, cc * csz2:(cc + 1) * csz2].rearrange(
                    "p (d k f) -> p d k f", d=GD, k=NHb))
            for d in range(GD):
                for c0 in range(0, C, 512):
                    c1 = min(C, c0 + 512)
                    ps_z = psz.tile([P, c1 - c0], F32, tag="psz")
                    for k in range(NHb):
                        nc.tensor.matmul(ps_z[:], w2t[:, d, k], h_tile[:, k, c0:c1],
                                         start=(k == 0), stop=(k == NHb - 1))
                    nc.vector.tensor_copy(o_tile[:, GD * cc + d, c0:c1], ps_z[:])
            if bi == last:
                # split the final store so most of it overlaps compute
                nc.scalar.dma_start(zt_a[:, GD * cc:GD * (cc + 1)],
                                    o_tile[:, GD * cc:GD * (cc + 1)])
        if bi != last:
            nc.scalar.dma_start(zt_a, o_tile[:])


def _route(xf, Wg):
    """Host gate: softmax over expert logits, top-2 (ties -> lower index,
    matching jax.lax.top_k)."""
    logits = xf @ Wg.T
    m = logits.max(axis=-1, keepdims=True)
    p = np.exp(logits - m)
    scores = p / p.sum(axis=-1, keepdims=True)
    i1 = scores.argmax(axis=-1)
    rows = np.arange(T)
    s1 = scores[rows, i1]
    masked = scores.copy()
    masked[rows, i1] = -np.inf
    i2 = masked.argmax(axis=-1)
    s2 = scores[rows, i2]
    return i1, s1 * ROUTE_SCALE, i2, s2 * ROUTE_SCALE


def _pack_w1(W1b, HB, npt, scale=1.0):
    """[D, 2*HB] -> [P, NHb*2*KD*P] flat per partition, (hc, half, ko, f).
    (For DoubleRow the ko axis is viewed as (ko//2, ko%2) — same layout.)"""
    NHb = HB // P
    Ar = (W1b * scale).reshape(KD, P, 2, NHb, P)   # [ko, ki, half, hc, f]
    return np.ascontiguousarray(
        Ar.transpose(1, 3, 2, 0, 4).reshape(P, NHb * 2 * KD * P).astype(npt))


def _pack_w2(W2b, HB, npt, scale=1.0):
    """[HB, D] -> [P, MD*NHb*P] flat per partition, (dc, ko, f)."""
    NHb = HB // P
    Br = (W2b * scale).reshape(NHb, P, MD, P)      # [ko, ki, dc, f]
    return np.ascontiguousarray(
        Br.transpose(1, 2, 0, 3).reshape(P, MD * NHb * P).astype(npt))


def _pack_x(xTb, C, npt=np.float16):
    """[D, n] -> [P, KD*C] (zero-pads the token dim to C)."""
    n = xTb.shape[1]
    out = np.zeros((P, KD, C), dtype=npt)
    out[:, :, :n] = xTb.reshape(KD, P, n).transpose(1, 0, 2).astype(npt)
    return out.reshape(P, KD * C)


def _r32(n):
    return max(32, -(-n // 32) * 32)


def prepare(x, Wg, W1, W2, Ws1, Ws2):
    """Host routing + per-core input maps.

    Returns (in_maps, toks, wts, assign, CA, CB) where assign[c] is the
    (expertA, expertB) pair owned by core c."""
    x = np.asarray(x, dtype=np.float32)
    Wg = np.asarray(Wg, dtype=np.float32)
    W1 = np.asarray(W1, dtype=np.float32)
    W2 = np.asarray(W2, dtype=np.float32)
    Ws1 = np.asarray(Ws1, dtype=np.float32)
    Ws2 = np.asarray(Ws2, dtype=np.float32)

    xf = np.ascontiguousarray(x.reshape(T, D))
    i1, s1, i2, s2 = _route(xf, Wg)

    toks, wts = [], []
    for e in range(E):
        sel = np.where((i1 == e) | (i2 == e))[0]
        toks.append(sel)
        wts.append(np.where(i1[sel] == e, s1[sel], s2[sel]).astype(np.float32))

    # Pair the 8 largest with the 8 smallest: slot A holds the big ones.
    order = sorted(range(E), key=lambda e: -len(toks[e]))
    assign = [(order[c], order[E - 1 - c]) for c in range(N_CORES)]
    CA = _r32(max(len(toks[a]) for a, _ in assign))
    CB = _r32(max(len(toks[b]) for _, b in assign))

    if ROUTED_FP8:
        npt_r, wscale = NP8, W8SCALE
    else:
        npt_r, wscale = NP16, 1.0
    npt_x = NP8 if (ROUTED_FP8 and DR_FC1) else NP16

    # Shared expert, split by hidden half: q=0 -> cols [0,HL), q=1 -> [HL,HS)
    ws1p, ws2p = [], []
    for q in range(2):
        W1s = np.concatenate(
            [Ws1[:, q * HL:(q + 1) * HL], Ws1[:, HS + q * HL:HS + (q + 1) * HL]],
            axis=1)
        ws1p.append(_pack_w1(W1s, HL, NP16))
        ws2p.append(_pack_w2(Ws2[q * HL:(q + 1) * HL], HL, NP16))

    in_maps = []
    for c in range(N_CORES):
        grp, q = c // 2, c % 2
        xd = _pack_x(np.ascontiguousarray(
            xf[grp * TDP2:(grp + 1) * TDP2].T), TDP2)
        im = {"ws1": ws1p[q], "ws2": ws2p[q], "xd": xd}
        for i, (e, cap) in enumerate(zip(assign[c], (CA, CB))):
            im[f"w1e{i}"] = _pack_w1(W1[e], H, npt_r, wscale)
            im[f"w2e{i}"] = _pack_w2(W2[e], H, npt_r, wscale)
            im[f"xg{i}"] = _pack_x(xf[toks[e]].T, cap, npt_x)
        in_maps.append(im)
    return in_maps, toks, wts, assign, CA, CB


def kernel(x, Wg, W1, W2, Ws1, Ws2):
    global LAST_RESULTS
    in_maps, toks, wts, assign, CA, CB = prepare(x, Wg, W1, W2, Ws1, Ws2)

    key = (CA, CB)
    if key not in _NC_CACHE:
        _NC_CACHE[key] = _build_nc(CA, CB)
    nc = _NC_CACHE[key]

    try:
        LAST_RESULTS = run_bass_kernel_spmd(nc, in_maps, list(range(N_CORES)))
    except Exception:
        # transient NRT device errors have been observed; retry once
        LAST_RESULTS = run_bass_kernel_spmd(nc, in_maps, list(range(N_CORES)))
    res = LAST_RESULTS.results

    zscale = 1.0 / (W8SCALE * W8SCALE) if ROUTED_FP8 else 1.0
    out = np.zeros((T, D), dtype=np.float32)
    for c in range(N_CORES):
        grp = c // 2
        for i, (e, cap) in enumerate(zip(assign[c], (CA, CB))):
            n = len(toks[e])
            z = res[c][f"zg{i}"].astype(np.float32).reshape(P, MD, cap)
            z = z.transpose(1, 0, 2).reshape(D, cap)[:, :n]
            out[toks[e]] += (zscale * wts[e])[:, None] * z.T
        zs = res[c]["zs"].astype(np.float32).reshape(P, MD, TDP2)
        out[grp * TDP2:(grp + 1) * TDP2] += zs.transpose(1, 0, 2).reshape(D, TDP2).T
    return out.reshape(B, S, D)
